# revision 1
# baseline (speedup 1.0000x reference)
"""Trainium2 Bass kernel for nn_Attention_42700564857309.

Multi-head attention (b=2, n=64*64=4096, dim=256, attn_dim=128, 4 heads,
head_dim=32) sharded over 8 NeuronCores as one (batch, head) pair per core;
the host sums the 4 per-head partial outputs per batch element (row-parallel
Wo split), so no collectives are needed.

Per-core device kernel. All layouts are chosen so no on-device transposes of
activations are ever needed; all matmuls run in float32r (single-pass fp32,
1 column/cycle at N>=256 vs 4 for plain fp32, ~1e-4 relative rounding):
  inputs:  xT = query_b^T [256, 4096], cT = context_b^T [256, 4096]
           (pre-transposed on host so the contraction dim is on partitions),
           wq/wk = head slice of Wq/Wk replicated `pack` times along columns,
           wv [256, 32], wo [32, 256]
  qT = wq.T @ xT -> [pack*32, 4096]: `pack` stacked replicas on partitions,
       so row-packed (tile_position) S matmuls can read per-row-group slices
  kT = wk.T @ cT -> [pack*32, 4096]
  v  = cT.T @ wv -> [4096, 32] + a ones column (-> 33 wide) so the PV matmul
       also produces softmax row sums in psum row 32 for free
  Attention per 512-wide i-chunk, in groups of `pack` j-tiles (128 keys):
    S^T[j,i] = kT_jt.T @ qT   K=32 matmuls row-packed via tile_position so
               `pack` of them run concurrently in the 128x128 PE array
    P^T = exp(scale*S^T)      one ScalarE op spanning the group's psum banks
                              (scores are ~N(0,1): max-subtraction unneeded)
    pv[0:33] += v_aug_jt.T @ P^T   f32r, accumulated over all 32 j-tiles
  Row sums are transposed to per-partition layout via a tiny DRAM round-trip
  DMA (cross-partition moves are DMA territory; a K=1 transpose-matmul
  faults the device and gpsimd partition_broadcast misreads partition-32
  sources); 1/rowsum is then folded into the PSUM->SBUF copy of the
  projected output as a per-partition tensor_scalar multiply.

Scheduling: the PE executes its queue in order, so S-matmul groups are
emitted `lead` groups ahead of their exp/PV consumers (3 S psum slots),
and the q/k/v projection units are interleaved into the attention stream
with deadline-based emission instead of running as a serial prologue.
ScalarE exp (~128us busy) is the roofline; measured ~220us/iteration
sustained on hardware (~2.9e-4 max relative error vs the fp32 reference).
"""

import contextlib

import numpy as np

import concourse.bacc as bacc
import concourse.mybir as mybir
import concourse.tile as tile
from concourse import bass_utils
from concourse.bass import ts

F32 = mybir.dt.float32
F32R = mybir.dt.float32r

B, HH, WW, C = 2, 64, 64, 256
N = HH * WW              # 4096
AD = 128                 # attn_dim
HEADS = 4
D = AD // HEADS          # 32 head dim
SCALE = float(D) ** -0.5
NCORES = 8

PACK = 3                 # row-packed S^T matmuls / exp group size (psum banks)
IC = 512                 # i-chunk width (one psum bank of fp32)
NIC = N // IC            # 8 i-chunks
JT = 128                 # j-tile height
NJT = N // JT            # 32 j-tiles
NIT = IC // JT           # 4 i-tiles per chunk
VW = D + 1               # v width incl. ones column

GROUPS = [PACK] * (NJT // PACK) + ([NJT % PACK] if NJT % PACK else [])


def build_program(mm_dt=F32R, proj_dt=F32R, n_ic=NIC, n_groups=None,
                  reps=1, loop_reps=None, pack=2, s_bufs=3, lead=2, pt_bufs=3, s_dt=None, tune=False, pv2=False,
                  skip_exp=False, skip_s=False, skip_pv=False, no_pack=False,
                  skip_indma=False):
    groups_all = [pack] * (NJT // pack) + ([NJT % pack] if NJT % pack else [])
    s_dt = mm_dt if s_dt is None else s_dt
    nc = bacc.Bacc("TRN2", target_bir_lowering=False, debug=False)

    IN_DT = proj_dt
    xT_d = nc.dram_tensor("xT", [C, N], IN_DT, kind="ExternalInput")
    cT_d = nc.dram_tensor("cT", [C, N], IN_DT, kind="ExternalInput")
    wq_d = nc.dram_tensor("wq", [C, PACK * D], IN_DT, kind="ExternalInput")
    wk_d = nc.dram_tensor("wk", [C, PACK * D], IN_DT, kind="ExternalInput")
    wv_d = nc.dram_tensor("wv", [C, D], IN_DT, kind="ExternalInput")
    wo_d = nc.dram_tensor("wo", [D, C], IN_DT, kind="ExternalInput")
    out_d = nc.dram_tensor("out", [N, C], F32, kind="ExternalOutput")

    with tile.TileContext(nc) as tc:
        with tc.tile_pool(name="big", bufs=1) as big, \
             tc.tile_pool(name="pt", bufs=pt_bufs) as ptp, \
             tc.tile_pool(name="att", bufs=3 if tune else 2) as attp, \
             tc.tile_pool(name="small", bufs=6 if tune else 4) as small, \
             tc.tile_pool(name="outp", bufs=4 if tune else 3) as outp, \
             tc.tile_pool(name="spsum", bufs=s_bufs, space="PSUM") as sps_p, \
             tc.tile_pool(name="pvpsum", bufs=1, space="PSUM") as pv_p, \
             tc.tile_pool(name="oppsum", bufs=1, space="PSUM") as op_p, \
             tc.tile_pool(name="dram", bufs=3 if tune else 2, space="DRAM") as dramp:

            loop_ctx = (tc.For_i(0, loop_reps, 1) if loop_reps
                        else contextlib.nullcontext())
            with loop_ctx:
              for _rep in range(reps):
                # ---- load inputs ---------------------------------------
                xT = big.tile([128, 2, N], IN_DT, tag="xT")
                cT = big.tile([128, 2, N], IN_DT, tag="cT")
                wq = big.tile([128, 2, PACK * D], IN_DT, tag="wq")
                wk = big.tile([128, 2, PACK * D], IN_DT, tag="wk")
                wv = big.tile([128, 2, D], IN_DT, tag="wv")
                wo = big.tile([96 if pv2 else D, C], IN_DT, tag="wo")
                ones = big.tile([128, 1], F32, tag="ones")
                HN = N // 2
                for cc in range(2):
                    nc.sync.dma_start(out=wq[:, cc, :],
                                      in_=wq_d.ap()[ts(cc, 128), :])
                    nc.sync.dma_start(out=wk[:, cc, :],
                                      in_=wk_d.ap()[ts(cc, 128), :])
                    nc.sync.dma_start(out=wv[:, cc, :],
                                      in_=wv_d.ap()[ts(cc, 128), :])
                    if not skip_indma:
                        QN = N // 4 if tune else HN
                        for q0 in range(0, HN, QN):
                            nc.sync.dma_start(
                                out=xT[:, cc, q0:q0 + QN],
                                in_=xT_d.ap()[ts(cc, 128), q0:q0 + QN])
                            nc.sync.dma_start(
                                out=cT[:, cc, q0:q0 + QN],
                                in_=cT_d.ap()[ts(cc, 128), q0:q0 + QN])
                for cc in range(2):
                    if not skip_indma:
                        QN = N // 4 if tune else HN
                        for q0 in range(HN, N, QN):
                            nc.sync.dma_start(
                                out=cT[:, cc, q0:q0 + QN],
                                in_=cT_d.ap()[ts(cc, 128), q0:q0 + QN])
                            nc.sync.dma_start(
                                out=xT[:, cc, q0:q0 + QN],
                                in_=xT_d.ap()[ts(cc, 128), q0:q0 + QN])
                nc.sync.dma_start(out=wo[0:D, :], in_=wo_d.ap())
                if pv2:
                    nc.sync.dma_start(out=wo[64:64 + D, :], in_=wo_d.ap())
                nc.vector.memset(ones[:], 1.0)
                if skip_exp or skip_s or skip_pv:
                    dummyf = big.tile([128, pack * IC], F32, tag="dummyf")
                    nc.vector.memset(dummyf[:], 0.5)
                    dummyr = big.tile([128, pack * IC], mm_dt, tag="dummyr")
                    nc.vector.tensor_copy(dummyr[:], dummyf[:])

                # ---- projection units (interleaved into attention) -----
                qT = big.tile([pack * D, N], s_dt, tag="qT")
                kT = big.tile([pack * D, N], s_dt, tag="kT")
                vsb = big.tile([128, NJT, VW], mm_dt, tag="vsb")
                for jt in range(NJT):                  # preset ones column
                    nc.vector.tensor_copy(vsb[:, jt, D:VW], ones[:])

                def emit_qT_unit(ic):
                    pq = op_p.tile([pack * D, IC], F32, tag="op", name="pq")
                    nc.tensor.matmul(pq[:], lhsT=wq[:, 0, 0:pack * D],
                                     rhs=xT[:, 0, ts(ic, IC)],
                                     start=True, stop=False)
                    nc.tensor.matmul(pq[:], lhsT=wq[:, 1, 0:pack * D],
                                     rhs=xT[:, 1, ts(ic, IC)],
                                     start=False, stop=True)
                    nc.vector.tensor_copy(qT[:, ts(ic, IC)], pq[:])

                def emit_kT_unit(ic):
                    pk = op_p.tile([pack * D, IC], F32, tag="op", name="pk")
                    nc.tensor.matmul(pk[:], lhsT=wk[:, 0, 0:pack * D],
                                     rhs=cT[:, 0, ts(ic, IC)],
                                     start=True, stop=False)
                    nc.tensor.matmul(pk[:], lhsT=wk[:, 1, 0:pack * D],
                                     rhs=cT[:, 1, ts(ic, IC)],
                                     start=False, stop=True)
                    nc.vector.tensor_copy(kT[:, ts(ic, IC)], pk[:])

                def emit_v_unit(g):
                    for jt in range(pack * g, min(pack * (g + 1), NJT)):
                        pvj = op_p.tile([128, D], F32, tag="op", name="pvj")
                        nc.tensor.matmul(pvj[:],
                                         lhsT=cT[:, 0, ts(jt, JT)],
                                         rhs=wv[:, 0, :],
                                         start=True, stop=False)
                        nc.tensor.matmul(pvj[:],
                                         lhsT=cT[:, 1, ts(jt, JT)],
                                         rhs=wv[:, 1, :],
                                         start=False, stop=True)
                        nc.vector.tensor_copy(vsb[:, jt, 0:D], pvj[:])

                # ---- attention main loop (software-pipelined) ----------
                glist = []
                gsel = groups_all if n_groups is None else groups_all[:n_groups]
                njt_used = sum(gsel)
                for ic in range(n_ic):
                    jt0 = 0
                    for gs in gsel:
                        glist.append((ic, jt0, gs))
                        jt0 += gs

                sp_t, pt_t, pv_t = {}, {}, {}
                att_t, rc_t = {}, {}
                pending = []

                def emit_S(k):
                    ic, jt0, gs = glist[k]
                    sp = sps_p.tile([128, pack * IC], F32, tag="s", name="sp")
                    sp_t[k] = sp
                    for t in range(gs):
                        if skip_s:
                            continue
                        if no_pack:
                            nc.tensor.matmul(
                                sp[:, ts(t, IC)],
                                lhsT=kT[0:D, ts(jt0 + t, JT)],
                                rhs=qT[0:D, ts(ic, IC)],
                                start=True, stop=True)
                        else:
                            nc.tensor.matmul(
                                sp[:, ts(t, IC)],
                                lhsT=kT[32 * t: 32 * t + D, ts(jt0 + t, JT)],
                                rhs=qT[32 * t: 32 * t + D, ts(ic, IC)],
                                start=True, stop=True,
                                tile_position=(32 * t, 0))

                def emit_exp(k):
                    ic, jt0, gs = glist[k]
                    sp = sp_t.pop(k)
                    pt = ptp.tile([128, pack * IC], mm_dt, tag="pt", name="pt")
                    pt_t[k] = pt
                    if not skip_exp:
                        nc.scalar.activation(
                            out=pt[:, 0: gs * IC],
                            in_=(dummyf if skip_s else sp)[:, 0: gs * IC],
                            func=mybir.ActivationFunctionType.Exp,
                            scale=SCALE)

                def finalize_dve(ic):
                    pv = pv_t.pop(ic)
                    AH = 97 if pv2 else VW
                    att = attp.tile([AH, IC], proj_dt, tag="att", name="att")
                    att_t[ic] = att
                    nc.vector.tensor_copy(att[:], (dummyf[0:AH, 0:IC] if skip_pv
                                                   else pv[0:AH, :]))
                    srow = dramp.tile([2, IC], F32, tag="srow")
                    nc.sync.dma_start(out=srow[0:1, :],
                                      in_=att[D:VW, :].bitcast(F32))
                    if pv2:
                        nc.sync.dma_start(out=srow[1:2, :],
                                          in_=att[96:97, :].bitcast(F32))
                    sumsT = small.tile([128, NIT], F32, tag="sumsT")
                    nc.sync.dma_start(
                        out=sumsT[:],
                        in_=srow[0:1, :].rearrange("one (t p) -> (one p) t",
                                                   p=JT))
                    rc = small.tile([128, NIT], F32, tag="rc", name="rc")
                    rc_t[ic] = rc
                    if pv2:
                        sumsT1 = small.tile([128, NIT], F32, tag="sumsT1",
                                            name="sumsT1")
                        nc.sync.dma_start(
                            out=sumsT1[:],
                            in_=srow[1:2, :].rearrange(
                                "one (t p) -> (one p) t", p=JT))
                        nc.vector.tensor_add(sumsT[:], sumsT[:], sumsT1[:])
                    nc.vector.reciprocal(rc[:], sumsT[:])
                    for t4 in range(NIT):
                        pending.append((ic, t4))

                def emit_PV(k):
                    ic, jt0, gs = glist[k]
                    if jt0 == 0:
                        pv_t[ic] = pv_p.tile([128, IC], F32, tag="pv", name="pv")
                    pv = pv_t[ic]
                    pt = pt_t.pop(k)
                    for t in range(gs):
                        if skip_pv:
                            continue
                        jt = jt0 + t
                        if pv2:
                            base = 64 * (jt % 2)
                            nc.tensor.matmul(
                                pv[base:base + VW, :],
                                lhsT=vsb[:, jt, :],
                                rhs=(dummyr if skip_exp else pt)[:, ts(t, IC)],
                                start=(jt == 0),
                                stop=(jt == njt_used - 1),
                                tile_position=(0, base))
                        else:
                            nc.tensor.matmul(
                                pv[0:VW, :],
                                lhsT=vsb[:, jt, :],
                                rhs=(dummyr if skip_exp else pt)[:, ts(t, IC)],
                                start=(jt == 0),
                                stop=(jt == njt_used - 1))
                    if jt0 + gs == njt_used:
                        finalize_dve(ic)

                ot_t = {}

                def emit_op(ic, t4):
                    att, rc = att_t[ic], rc_t[ic]
                    op = op_p.tile([128, IC], F32, tag="op", name="op")
                    nc.tensor.matmul(op[:, 0:C],
                                     lhsT=att[0:D, ts(t4, JT)],
                                     rhs=wo[0:D, :],
                                     start=True, stop=not pv2)
                    if pv2:
                        nc.tensor.matmul(op[:, 0:C],
                                         lhsT=att[64:96, ts(t4, JT)],
                                         rhs=wo[64:96, :],
                                         start=False, stop=True,
                                         tile_position=(64, 0))
                    if t4 == 0:
                        ot_t[ic] = outp.tile([128, NIT, C], F32, tag="ot",
                                             name="ot")
                    ot = ot_t[ic]
                    nc.vector.tensor_scalar_mul(ot[:, t4, :], op[:, 0:C],
                                                rc[:, t4:t4 + 1])
                    if t4 == NIT - 1:
                        # one DMA for the whole 512-row chunk; HBM rows
                        # ic*512 + t4*128 + p  <-  sbuf [p, t4, :]
                        dst = out_d.ap()[ic * IC:(ic + 1) * IC, :].rearrange(
                            "(t p) c -> p t c", p=JT)
                        nc.sync.dma_start(out=dst, in_=ot_t.pop(ic)[:])

                nvu = (njt_used + pack - 1) // pack       # v proj units
                nku = (njt_used * JT + IC - 1) // IC      # kT proj units
                if glist:
                    emit_qT_unit(0)
                    emit_kT_unit(0)
                    emit_v_unit(0)
                    qT_done, kT_done, v_done = 1, 1, 1
                    for j in range(min(lead, len(glist))):
                        emit_S(j)
                    for k in range(len(glist)):
                        j = k + lead
                        if j < len(glist):
                            icj, jt0j, gsj = glist[j]
                            for la in (j, j + 1):
                                if la < len(glist) and glist[la][1] == 0 \
                                        and qT_done <= glist[la][0] < n_ic:
                                    emit_qT_unit(qT_done)
                                    qT_done += 1
                            need_k = min(((jt0j + gsj) * JT + IC - 1) // IC,
                                         nku) if icj == 0 else nku
                            while kT_done < need_k:
                                emit_kT_unit(kT_done)
                                kT_done += 1
                            gidx = (k + 2) if icj == 0 else nvu
                            while v_done < min(gidx, nvu):
                                emit_v_unit(v_done)
                                v_done += 1
                            emit_S(j)
                        emit_exp(k)
                        emit_PV(k)
                        if pending:
                            emit_op(*pending.pop(0))
                    while pending:
                        emit_op(*pending.pop(0))

    nc.compile()
    return nc


_CACHE = {}


def get_program():
    if "nc" not in _CACHE:
        _CACHE["nc"] = build_program()
    return _CACHE["nc"]


def make_in_maps(query, context, Wq, Wk, Wv, Wo):
    q = np.ascontiguousarray(
        np.asarray(query, dtype=np.float32).reshape(B, N, C).transpose(0, 2, 1))
    c = np.ascontiguousarray(
        np.asarray(context, dtype=np.float32).reshape(B, N, C).transpose(0, 2, 1))
    Wq = np.asarray(Wq, dtype=np.float32)
    Wk = np.asarray(Wk, dtype=np.float32)
    Wv = np.asarray(Wv, dtype=np.float32)
    Wo = np.asarray(Wo, dtype=np.float32)
    in_maps = []
    for core in range(NCORES):
        b, h = divmod(core, HEADS)
        in_maps.append({
            "xT": q[b],
            "cT": c[b],
            "wq": np.ascontiguousarray(
                np.tile(Wq[:, h * D:(h + 1) * D], (1, PACK))),
            "wk": np.ascontiguousarray(
                np.tile(Wk[:, h * D:(h + 1) * D], (1, PACK))),
            "wv": np.ascontiguousarray(Wv[:, h * D:(h + 1) * D]),
            "wo": np.ascontiguousarray(Wo[h * D:(h + 1) * D, :]),
        })
    return in_maps


def combine(results):
    out = np.zeros((B, N, C), np.float32)
    for core in range(NCORES):
        b = core // HEADS
        out[b] += results[core]["out"]
    return out.reshape(B, HH, WW, C)


def kernel(query, context, Wq, Wk, Wv, Wo):
    nc = get_program()
    in_maps = make_in_maps(query, context, Wq, Wk, Wv, Wo)
    res = bass_utils.run_bass_kernel_spmd(nc, in_maps,
                                          core_ids=list(range(NCORES)))
    return combine(res.results)



# revision 9
# speedup vs baseline: 2.3758x; 2.3758x over previous
"""Trainium2 Bass kernel for nn_Attention_42700564857309.

Multi-head attention (b=2, n=64*64=4096, dim=256, attn_dim=128, 4 heads,
head_dim=32) sharded over 8 NeuronCores as one (batch, head) pair per core;
the host sums the 4 per-head partial outputs per batch element (row-parallel
Wo split), so no collectives are needed.

v2 design. The softmax exp of the 4096x4096 score matrix (16.7M elements
per core) is the roofline: ScalarE's activation LUT is the only exact-exp
engine (1 elem/cycle/lane @1.2GHz ~= 117us). To beat that, exp work is
SPLIT between two engines, alternating per 3-j-tile group:
  - ScalarE groups: exact exp via activation (scale folded in).
  - VectorE groups: Schraudolph fast exp: one tensor_scalar op computing
    int32(round(s * SCALE*2^23/ln2 + (127<<23 - C))), bit-reinterpreted as
    f32 (~ +-3% per-element, C tuned; final output error ~9e-3 with 33/88
    groups offloaded, vs the 2e-2 gate; errors average out in softmax).
Both engines read different PSUM banks concurrently (legal on TRN2).

Layouts (all matmuls f32r single-pass; no on-device activations transposes):
  xT/cT [256, 4096] pre-transposed on host; wq/wk head slices replicated
  3x along columns so pack=3 row-packed (tile_position) S matmuls use 96
  of 128 PE rows concurrently; v is stored [key, 0:32]=v, col 32 = ones,
  cols 33:64 = zeros so PV matmuls with M=64 col-tiles are ISA-legal:
  even j-tiles accumulate into pv[0:64] (col groups 0-1), odd into
  pv[64:128] (col groups 2-3) -- the two chains run concurrently in the
  PE array, halving PV streaming time. Softmax row sums appear for free
  at pv rows 32 (even) and 96 (odd).
Row sums are transposed to per-partition layout via a small DRAM
round-trip DMA; the reciprocal is DEFERRED ~2 groups so the DMA latency
never head-of-line-blocks the VectorE queue (a major stall in v1), then
folded into the PSUM->SBUF copy of the projected output as a
per-partition tensor_scalar multiply.

Scheduling: S-matmul groups run `lead` groups ahead of their exp
consumers (2 S psum buffers of 3 banks each; 6+1+1 = 8 PSUM banks), and
the q/k/v projection units are emitted just-in-time relative to their
S/PV consumers (also fixing a v1 race where prologue S groups could read
kT columns whose producing unit sat behind them in the PE queue).
"""

import contextlib

import numpy as np

import concourse.bacc as bacc
import concourse.mybir as mybir
import concourse.tile as tile
from concourse import bass_utils
from concourse.bass import ts

F32 = mybir.dt.float32
F32R = mybir.dt.float32r
BF16 = mybir.dt.bfloat16
I16 = mybir.dt.int16

B, HH, WW, C = 2, 64, 64, 256
N = HH * WW              # 4096
AD = 128                 # attn_dim
HEADS = 4
D = AD // HEADS          # 32 head dim
SCALE = float(D) ** -0.5
NCORES = 8

PACK = 3                 # row-packed S^T matmuls / exp group size (psum banks)
IC = 512                 # i-chunk width (one psum bank of fp32)
NIC = N // IC            # 8 i-chunks
JT = 128                 # j-tile height
NJT = N // JT            # 32 j-tiles
NIT = IC // JT           # 4 i-tiles per chunk
VW = 64                  # v width: 32 v + 1 ones + 31 zeros (M=64 col tile)

# Schraudolph fast-exp constants: bf16 result built as
# int16(round(SCALE*s * 2^7/ln2 + (127<<7) - C16)) reinterpreted as bf16
EXP_A = float(2.0 ** 7 / np.log(2.0)) * SCALE
EXP_B = float((127 << 7) - 5.5)

GROUPS = [PACK] * (NJT // PACK) + ([NJT % PACK] if NJT % PACK else [])


def build_program(mm_dt=F32R, pt_dt=BF16, proj_dt=F32R, n_ic=NIC, n_groups=None,
                  loop_reps=None, pack=PACK, s_bufs=2, lead=2, pt_bufs=3,
                  n_dve=33, pv2=True, rc_delay=2, v_sub=8,
                  skip_exp=False, skip_s=False, skip_pv=False,
                  skip_indma=False, skip_out=False, out2=True, rc_one=False):
    groups_all = [pack] * (NJT // pack) + ([NJT % pack] if NJT % pack else [])
    nc = bacc.Bacc("TRN2", target_bir_lowering=False, debug=False)

    IN_DT = proj_dt
    xT_d = nc.dram_tensor("xT", [C, N], IN_DT, kind="ExternalInput")
    cT_d = nc.dram_tensor("cT", [C, N], IN_DT, kind="ExternalInput")
    wq_d = nc.dram_tensor("wq", [C, PACK * D], IN_DT, kind="ExternalInput")
    wk_d = nc.dram_tensor("wk", [C, PACK * D], IN_DT, kind="ExternalInput")
    wv_d = nc.dram_tensor("wv", [C, D], IN_DT, kind="ExternalInput")
    wo_d = nc.dram_tensor("wo", [D, C], IN_DT, kind="ExternalInput")
    out_d = nc.dram_tensor("out", [N, C], F32, kind="ExternalOutput")

    AH = 97 if pv2 else D + 1     # att rows needed (v + rowsum rows)

    with tile.TileContext(nc) as tc:
        with tc.tile_pool(name="big", bufs=1) as big, \
             tc.tile_pool(name="pt", bufs=pt_bufs) as ptp, \
             tc.tile_pool(name="att", bufs=2) as attp, \
             tc.tile_pool(name="small", bufs=4) as small, \
             tc.tile_pool(name="outp", bufs=3) as outp, \
             tc.tile_pool(name="spsum", bufs=s_bufs, space="PSUM") as sps_p, \
             tc.tile_pool(name="pvpsum", bufs=1, space="PSUM") as pv_p, \
             tc.tile_pool(name="oppsum", bufs=1, space="PSUM") as op_p, \
             tc.tile_pool(name="dram", bufs=2, space="DRAM") as dramp:

            # vsb: [key_in_tile, jt, 0:32]=v, col 32 = ones, 33: = zeros.
            # The pad/ones cols are written once, outside the hw loop (the
            # per-rep v-copies only touch cols 0:32).
            vsb = big.tile([128, NJT, VW], pt_dt, tag="vsb")
            nc.vector.memset(vsb[:, :, D:VW], 0.0)
            nc.vector.memset(vsb[:, :, D:D + 1], 1.0)
            # wo padded to att height; rows 32, 33:63, 96 stay zero so the
            # out projection is ONE K=AH matmul (rowsum/pad rows hit zeros)
            wo = big.tile([AH, C], IN_DT, tag="wo")
            nc.vector.memset(wo[:].bitcast(F32), 0.0)

            loop_ctx = (tc.For_i(0, loop_reps, 1) if loop_reps
                        else contextlib.nullcontext())
            with loop_ctx:
                # ---- load inputs ---------------------------------------
                xT = big.tile([128, 2, N], IN_DT, tag="xT")
                cT = big.tile([128, 2, N], IN_DT, tag="cT")
                wq = big.tile([128, 2, PACK * D], IN_DT, tag="wq")
                wk = big.tile([128, 2, PACK * D], IN_DT, tag="wk")
                wv = big.tile([128, 2, D], IN_DT, tag="wv")
                HN = N // 2
                for cc in range(2):
                    nc.sync.dma_start(out=wq[:, cc, :],
                                      in_=wq_d.ap()[ts(cc, 128), :])
                    nc.sync.dma_start(out=wk[:, cc, :],
                                      in_=wk_d.ap()[ts(cc, 128), :])
                    nc.sync.dma_start(out=wv[:, cc, :],
                                      in_=wv_d.ap()[ts(cc, 128), :])
                    if not skip_indma:
                        for q0 in range(0, HN, HN):
                            nc.sync.dma_start(
                                out=xT[:, cc, q0:q0 + HN],
                                in_=xT_d.ap()[ts(cc, 128), q0:q0 + HN])
                            nc.sync.dma_start(
                                out=cT[:, cc, q0:q0 + HN],
                                in_=cT_d.ap()[ts(cc, 128), q0:q0 + HN])
                for cc in range(2):
                    if not skip_indma:
                        for q0 in range(HN, N, HN):
                            nc.sync.dma_start(
                                out=cT[:, cc, q0:q0 + HN],
                                in_=cT_d.ap()[ts(cc, 128), q0:q0 + HN])
                            nc.sync.dma_start(
                                out=xT[:, cc, q0:q0 + HN],
                                in_=xT_d.ap()[ts(cc, 128), q0:q0 + HN])
                nc.sync.dma_start(out=wo[0:D, :], in_=wo_d.ap())
                if pv2:
                    nc.sync.dma_start(out=wo[64:64 + D, :], in_=wo_d.ap())
                if skip_exp or skip_s or skip_pv:
                    dummyf = big.tile([128, pack * IC], F32, tag="dummyf")
                    nc.vector.memset(dummyf[:], 0.5)
                    dummyr = big.tile([128, pack * IC], pt_dt, tag="dummyr")
                    nc.vector.tensor_copy(dummyr[:], dummyf[:])

                # ---- projection units (emitted just-in-time) -----------
                qT = big.tile([pack * D, N], mm_dt, tag="qT")
                kT = big.tile([pack * D, N], mm_dt, tag="kT")

                def emit_qT_unit(ic):
                    pq = op_p.tile([pack * D, IC], F32, tag="op", name="pq")
                    nc.tensor.matmul(pq[:], lhsT=wq[:, 0, 0:pack * D],
                                     rhs=xT[:, 0, ts(ic, IC)],
                                     start=True, stop=False)
                    nc.tensor.matmul(pq[:], lhsT=wq[:, 1, 0:pack * D],
                                     rhs=xT[:, 1, ts(ic, IC)],
                                     start=False, stop=True)
                    nc.vector.tensor_copy(qT[:, ts(ic, IC)], pq[:])

                def emit_kT_unit(u):
                    pk = op_p.tile([pack * D, IC], F32, tag="op", name="pk")
                    nc.tensor.matmul(pk[:], lhsT=wk[:, 0, 0:pack * D],
                                     rhs=cT[:, 0, ts(u, IC)],
                                     start=True, stop=False)
                    nc.tensor.matmul(pk[:], lhsT=wk[:, 1, 0:pack * D],
                                     rhs=cT[:, 1, ts(u, IC)],
                                     start=False, stop=True)
                    nc.vector.tensor_copy(kT[:, ts(u, IC)], pk[:])

                def emit_v_unit(h):
                    # batch v_sub j-tiles: 2*v_sub matmuls into one psum
                    # tile, one strided copy out
                    jt0, jt1 = v_sub * h, min(v_sub * (h + 1), NJT)
                    pvj = op_p.tile([128, (jt1 - jt0) * D], F32, tag="op",
                                    name="pvj")
                    for jt in range(jt0, jt1):
                        o = (jt - jt0) * D
                        nc.tensor.matmul(pvj[:, o:o + D],
                                         lhsT=cT[:, 0, ts(jt, JT)],
                                         rhs=wv[:, 0, :],
                                         start=True, stop=False)
                        nc.tensor.matmul(pvj[:, o:o + D],
                                         lhsT=cT[:, 1, ts(jt, JT)],
                                         rhs=wv[:, 1, :],
                                         start=False, stop=True)
                    nc.vector.tensor_copy(vsb[:, jt0:jt1, 0:D], pvj[:])

                nqu, nku = n_ic, 0
                qT_done = kT_done = v_done = 0

                def ensure_qT(ic):
                    nonlocal qT_done
                    while qT_done <= min(ic, n_ic - 1):
                        emit_qT_unit(qT_done)
                        qT_done += 1

                def ensure_kT(cols):
                    nonlocal kT_done
                    need = min((cols + IC - 1) // IC, N // IC)
                    while kT_done < need:
                        emit_kT_unit(kT_done)
                        kT_done += 1

                def ensure_v(jt_max):
                    nonlocal v_done
                    need = min((jt_max + v_sub - 1) // v_sub,
                               (NJT + v_sub - 1) // v_sub)
                    while v_done < need:
                        emit_v_unit(v_done)
                        v_done += 1

                # ---- attention main loop (software-pipelined) ----------
                glist = []
                gsel = groups_all if n_groups is None else groups_all[:n_groups]
                njt_used = sum(gsel)
                for ic in range(n_ic):
                    jt0 = 0
                    for gs in gsel:
                        glist.append((ic, jt0, gs))
                        jt0 += gs

                # Bresenham spread of the DVE fast-exp groups
                eng = []
                acc = 0
                for k in range(len(glist)):
                    acc += n_dve
                    if acc >= len(glist):
                        acc -= len(glist)
                        eng.append("dve")
                    else:
                        eng.append("sc")

                sp_t, pt_t, pv_t = {}, {}, {}
                att_t, rc_t, srow_t = {}, {}, {}
                pending = []

                def emit_S(j):
                    ic, jt0, gs = glist[j]
                    ensure_qT(ic)
                    ensure_kT((jt0 + gs) * JT if ic == 0 else N)
                    sp = sps_p.tile([128, pack * IC], F32, tag="s", name="sp")
                    sp_t[j] = sp
                    for t in range(gs):
                        if skip_s:
                            continue
                        nc.tensor.matmul(
                            sp[:, ts(t, IC)],
                            lhsT=kT[32 * t: 32 * t + D, ts(jt0 + t, JT)],
                            rhs=qT[32 * t: 32 * t + D, ts(ic, IC)],
                            start=True, stop=True,
                            tile_position=(32 * t, 0))

                def emit_exp(k):
                    ic, jt0, gs = glist[k]
                    sp = sp_t.pop(k)
                    pt = ptp.tile([128, pack * IC], pt_dt, tag="pt", name="pt")
                    pt_t[k] = pt
                    if skip_exp:
                        return
                    src = dummyf if skip_s else sp
                    if eng[k] == "sc":
                        nc.scalar.activation(
                            out=pt[:, 0: gs * IC],
                            in_=src[:, 0: gs * IC],
                            func=mybir.ActivationFunctionType.Exp,
                            scale=SCALE)
                    else:
                        nc.vector.tensor_scalar(
                            out=pt[:, 0: gs * IC].bitcast(I16),
                            in0=src[:, 0: gs * IC],
                            scalar1=EXP_A, scalar2=EXP_B,
                            op0=mybir.AluOpType.mult,
                            op1=mybir.AluOpType.add)

                def finalize(ic):
                    pv = pv_t.pop(ic)
                    att = attp.tile([AH, IC], mm_dt, tag="att", name="att")
                    att_t[ic] = att
                    nc.vector.tensor_copy(att[:], (dummyf[0:AH, 0:IC] if skip_pv
                                                   else pv[0:AH, :]))
                    srow = dramp.tile([2, IC], F32, tag="srow")
                    nc.sync.dma_start(out=srow[0:1, :],
                                      in_=att[D:D + 1, :].bitcast(F32))
                    if pv2:
                        nc.sync.dma_start(out=srow[1:2, :],
                                          in_=att[96:97, :].bitcast(F32))
                    sumsT = small.tile([128, 2, NIT], F32, tag="sumsT")
                    nc.sync.dma_start(
                        out=sumsT[:, 0, :],
                        in_=srow[0:1, :].rearrange("one (t p) -> (one p) t",
                                                   p=JT))
                    if pv2:
                        nc.sync.dma_start(
                            out=sumsT[:, 1, :],
                            in_=srow[1:2, :].rearrange(
                                "one (t p) -> (one p) t", p=JT))
                    srow_t[ic] = sumsT

                def emit_rc(ic):
                    # deferred: by now the sumsT DMAs have had ~rc_delay
                    # groups of latency budget
                    sumsT = srow_t.pop(ic)
                    rc = small.tile([128, NIT], F32, tag="rc", name="rc")
                    rc_t[ic] = rc
                    if rc_one:
                        nc.vector.memset(rc[:], 1.0)
                        return
                    if pv2:
                        nc.vector.tensor_add(sumsT[:, 0, :], sumsT[:, 0, :],
                                             sumsT[:, 1, :])
                    nc.vector.reciprocal(rc[:], sumsT[:, 0, :])

                def emit_PV(k):
                    ic, jt0, gs = glist[k]
                    ensure_v(jt0 + gs if ic == 0 else NJT)
                    if jt0 == 0:
                        pv_t[ic] = pv_p.tile([128, IC], F32, tag="pv",
                                             name="pv")
                    pv = pv_t[ic]
                    pt = pt_t.pop(k)
                    for t in range(gs):
                        if skip_pv:
                            continue
                        jt = jt0 + t
                        rhs = (dummyr if skip_exp else pt)[:, ts(t, IC)]
                        if pv2:
                            base = 64 * (jt % 2)
                            nc.tensor.matmul(
                                pv[base:base + 64, :],
                                lhsT=vsb[:, jt, 0:64],
                                rhs=rhs,
                                start=(jt <= 1),
                                stop=(jt == njt_used - 1),
                                tile_position=(0, base))
                        else:
                            nc.tensor.matmul(
                                pv[0:D + 1, :],
                                lhsT=vsb[:, jt, 0:D + 1],
                                rhs=rhs,
                                start=(jt == 0),
                                stop=(jt == njt_used - 1))
                    if jt0 + gs == njt_used:
                        finalize(ic)
                        for t4 in range(NIT):
                            pending.append((k + rc_delay, ic, t4))

                ot_t = {}

                def emit_op(ic, t4):
                    if skip_out:
                        return
                    if ic not in rc_t:
                        emit_rc(ic)
                    att, rc = att_t[ic], rc_t[ic]
                    op = op_p.tile([128, IC], F32, tag="op", name="op")
                    nc.tensor.matmul(op[:, 0:C],
                                     lhsT=att[0:AH, ts(t4, JT)],
                                     rhs=wo[0:AH, :],
                                     start=True, stop=True)
                    if t4 == 0:
                        ot_t[ic] = outp.tile([128, NIT, C], F32, tag="ot",
                                             name="ot")
                    ot = ot_t[ic]
                    nc.vector.tensor_scalar_mul(ot[:, t4, :], op[:, 0:C],
                                                rc[:, t4:t4 + 1])
                    if t4 == NIT - 1:
                        # one DMA for the whole 512-row chunk; HBM rows
                        # ic*512 + t4*128 + p  <-  sbuf [p, t4, :]
                        dst = out_d.ap()[ic * IC:(ic + 1) * IC, :].rearrange(
                            "(t p) c -> p t c", p=JT)
                        nc.sync.dma_start(out=dst, in_=ot_t.pop(ic)[:])

                if glist:
                    for j in range(min(lead, len(glist))):
                        emit_S(j)
                    for k in range(len(glist)):
                        j = k + lead
                        if j < len(glist):
                            emit_S(j)
                        emit_exp(k)
                        emit_PV(k)
                        if pending and pending[0][0] <= k:
                            _, ic_, t4_ = pending.pop(0)
                            emit_op(ic_, t4_)
                    while pending:
                        _, ic_, t4_ = pending.pop(0)
                        emit_op(ic_, t4_)

    nc.compile()
    return nc


_CACHE = {}


def get_program():
    if "nc" not in _CACHE:
        _CACHE["nc"] = build_program()
    return _CACHE["nc"]


def make_in_maps(query, context, Wq, Wk, Wv, Wo):
    q = np.ascontiguousarray(
        np.asarray(query, dtype=np.float32).reshape(B, N, C).transpose(0, 2, 1))
    c = np.ascontiguousarray(
        np.asarray(context, dtype=np.float32).reshape(B, N, C).transpose(0, 2, 1))
    Wq = np.asarray(Wq, dtype=np.float32)
    Wk = np.asarray(Wk, dtype=np.float32)
    Wv = np.asarray(Wv, dtype=np.float32)
    Wo = np.asarray(Wo, dtype=np.float32)
    in_maps = []
    for core in range(NCORES):
        b, h = divmod(core, HEADS)
        in_maps.append({
            "xT": q[b],
            "cT": c[b],
            "wq": np.ascontiguousarray(
                np.tile(Wq[:, h * D:(h + 1) * D], (1, PACK))),
            "wk": np.ascontiguousarray(
                np.tile(Wk[:, h * D:(h + 1) * D], (1, PACK))),
            "wv": np.ascontiguousarray(Wv[:, h * D:(h + 1) * D]),
            "wo": np.ascontiguousarray(Wo[h * D:(h + 1) * D, :]),
        })
    return in_maps


def combine(results):
    out = np.zeros((B, N, C), np.float32)
    for core in range(NCORES):
        b = core // HEADS
        out[b] += results[core]["out"]
    return out.reshape(B, HH, WW, C)


def kernel(query, context, Wq, Wk, Wv, Wo):
    nc = get_program()
    in_maps = make_in_maps(query, context, Wq, Wk, Wv, Wo)
    res = bass_utils.run_bass_kernel_spmd(nc, in_maps,
                                          core_ids=list(range(NCORES)))
    return combine(res.results)
